# revision 10
# baseline (speedup 1.0000x reference)
"""DKVMN forward kernel for 8 Trainium2 NeuronCores (Bass/Tile).

Self-contained: takes the full un-sharded inputs of the reference
`setup_inputs()`, shards batch-parallel across 8 cores, runs one Bass/Tile
program per core, and assembles the reference's 4 outputs.
"""
import sys

sys.path.insert(0, '/opt/trn_rl_repo')

from contextlib import ExitStack

import numpy as np

import concourse.bacc as bacc
import concourse.bass as bass
import concourse.tile as tile
from concourse import mybir
from concourse.bass_utils import run_bass_kernel_spmd
from concourse.masks import make_identity

F = mybir.dt.float32
I32 = mybir.dt.int32
AF = mybir.ActivationFunctionType
OP = mybir.AluOpType

B, L, K, D, M = 64, 511, 4, 64, 50
T = L + 1            # 512
NCORE = 8
BL = B // NCORE      # 8 batch elements per core
NCHUNK = (M * D) // 128   # 25
EPS = 1e-12

# how many of the 25 chunks per batch element run entirely on GPSIMD
# (load-balancing the DVE); the rest run we/scan/tmat on DVE with beta on GP
GP_CHUNKS = 0


def host_prep(inputs):
    f32 = np.float32
    qseqs = np.asarray(inputs['qseqs']); cseqs = np.asarray(inputs['cseqs'])
    rseqs = np.asarray(inputs['rseqs'])
    sq = np.asarray(inputs['shft_qseqs']); sc = np.asarray(inputs['shft_cseqs'])
    sr = np.asarray(inputs['shft_rseqs'])
    cq = np.concatenate([qseqs[:, :1], sq], 1).astype(np.int64)
    cc = np.concatenate([cseqs[:, :1, :], sc], 1).astype(np.int64)
    cr = np.concatenate([rseqs[:, :1], sr], 1)
    c_num = np.maximum((cc >= 0).sum(-1), 1)
    inv_c = (1.0 / c_num).astype(f32)
    r = cr.astype(f32)

    Wk = np.asarray(inputs['Wk'], f32); bk = np.asarray(inputs['bk'], f32)
    Wv = np.asarray(inputs['Wv'], f32); bv = np.asarray(inputs['bv'], f32)
    Mk = np.asarray(inputs['Mk'], f32); Mv0 = np.asarray(inputs['Mv0'], f32)
    We = np.asarray(inputs['We'], f32); be = np.asarray(inputs['be'], f32)
    Wa = np.asarray(inputs['Wa'], f32); ba = np.asarray(inputs['ba'], f32)
    Wf = np.asarray(inputs['Wf'], f32); bf = np.asarray(inputs['bf'], f32)
    Wp = np.asarray(inputs['Wp'], f32); bp = np.asarray(inputs['bp'], f32)
    Wq = np.asarray(inputs['Wq'], f32); bq = np.asarray(inputs['bq'], f32)
    Ws = np.asarray(inputs['Ws'], f32); bs = np.asarray(inputs['bs'], f32)
    c_tab = np.ascontiguousarray(np.asarray(inputs['c_emb_table'], f32))
    q_tab = np.ascontiguousarray(np.asarray(inputs['q_emb_table'], f32))
    const = np.float32(np.asarray(inputs['constant']))
    NQ = q_tab.shape[0]

    SelW = np.zeros((M, NCHUNK * 128), f32)
    for ci in range(NCHUNK):
        for half in range(2):
            SelW[2 * ci + half,
                 ci * 128 + half * 64: ci * 128 + (half + 1) * 64] = 1.0
    E64 = np.zeros((64, 128), f32)
    for p in range(128):
        E64[p % 64, p] = 1.0

    shared = dict(
        q_tab=q_tab, c_tab=c_tab,
        Wk=Wk, Wv_bot=np.ascontiguousarray(Wv[2 * D:]),
        Wv_dif=np.ascontiguousarray(Wv[:2 * D] - Wv[2 * D:]).astype(f32),
        MkT=np.ascontiguousarray(Mk.T), We=We, Wa=Wa,
        Wf_top=np.ascontiguousarray(Wf[:D]),
        Wf_bot=np.ascontiguousarray(Wf[D:]),
        lhsT_p=np.vstack([Wp, (bp + bq + bs)[None]]).astype(f32),
        Ws_h=Ws, Wq_h=Wq,
        lhsT_qq=np.vstack([Wq, (bq + 2.0 * const)[None]]).astype(f32),
        biases=np.stack([bk, bv, be, ba, bf], 1).astype(f32),
        Mv0c=np.ascontiguousarray(Mv0.reshape(M * D).reshape(NCHUNK, 128).T),
        SelW=SelW, E64=E64,
        E64T=np.ascontiguousarray(E64.T),
        ones128=np.ones((128, 1), f32),
        ones1=np.ones((1, 128), f32),
    )

    per_core = []
    for c in range(NCORE):
        bsl = slice(c * BL, (c + 1) * BL)
        cq_c, cc_c = cq[bsl], cc[bsl]
        sr_c = np.asarray(sr[bsl])
        idx_q = np.zeros((BL, 128, 4), np.int32)
        idx_c = np.zeros((BL, 128, 16), np.int32)
        for b in range(BL):
            for j in range(4):
                toks = np.arange(128) + 128 * j
                idx_q[b, :, j] = cq_c[b, toks]
                for k in range(K):
                    idx_c[b, :, k * 4 + j] = cc_c[b, toks, k] + 1
        s_all = np.zeros((128, 32), f32)
        mask = np.zeros((128, 32), f32)
        invc_col = np.zeros((BL, 128, 4), f32)
        for b in range(BL):
            for j in range(4):
                toks = np.arange(128) + 128 * j
                valid = toks >= 1
                lab = np.zeros(128, np.int64)
                lab[valid] = sr_c[b, toks[valid] - 1]
                s_all[:, b * 4 + j] = 2.0 * lab - 1.0
                mask[:, b * 4 + j] = valid.astype(f32)
                invc_col[b, :, j] = inv_c[c * BL + b, toks]
        per_core.append(dict(
            idx_q=idx_q, idx_c=idx_c,
            r_row=r[bsl].astype(f32), invc_col=invc_col,
            s_all=s_all, mask_all=mask,
        ))

    t_out = sr.reshape(-1).astype(np.int32)
    qf_out = (np.asarray(sq).reshape(-1).astype(np.int64)
              + NQ * sr.reshape(-1).astype(np.int64)).astype(np.int32)
    return shared, per_core, t_out, qf_out


SHARED_SPECS = [
    ('q_tab', None, F), ('c_tab', None, F),
    ('Wk', [2 * D, D], F), ('Wv_bot', [2 * D, D], F), ('Wv_dif', [2 * D, D], F),
    ('MkT', [D, M], F), ('We', [D, D], F), ('Wa', [D, D], F),
    ('Wf_top', [D, D], F), ('Wf_bot', [D, D], F),
    ('lhsT_p', [D + 1, 2], F), ('Ws_h', [D, 2], F), ('Wq_h', [D, 2], F),
    ('lhsT_qq', [D + 1, 2], F),
    ('biases', [D, 5], F), ('Mv0c', [128, NCHUNK], F),
    ('SelW', [M, NCHUNK * 128], F), ('E64', [D, 128], F), ('E64T', [128, D], F),
    ('ones128', [128, 1], F), ('ones1', [1, 128], F),
]
PER_CORE_SPECS = [
    ('idx_q', [BL, 128, 4], I32), ('idx_c', [BL, 128, 16], I32),
    ('r_row', [BL, T], F), ('invc_col', [BL, 128, 4], F),
    ('s_all', [128, 32], F), ('mask_all', [128, 32], F),
]


def build_program(ctx: ExitStack, tc: tile.TileContext, dram):
    nc = tc.nc

    consts = ctx.enter_context(tc.tile_pool(name='consts', bufs=1))
    sbA = ctx.enter_context(tc.tile_pool(name='sbA', bufs=2))
    sbB = ctx.enter_context(tc.tile_pool(name='sbB', bufs=3))
    sbD = ctx.enter_context(tc.tile_pool(name='sbD', bufs=1))
    psA = ctx.enter_context(tc.tile_pool(name='psA', bufs=2, space='PSUM'))
    psW = ctx.enter_context(tc.tile_pool(name='psW', bufs=2, space='PSUM'))
    psR = ctx.enter_context(tc.tile_pool(name='psR', bufs=2, space='PSUM'))

    # ---- load constants/weights to SBUF
    cs = {}
    for name, shape, dt in SHARED_SPECS:
        if name in ('q_tab', 'c_tab'):
            continue
        t_ = consts.tile(shape, dt, tag=f'c_{name}')
        nc.sync.dma_start(t_[:], dram[name][:])
        cs[name] = t_
    for name, shape, dt in PER_CORE_SPECS:
        if name in ('idx_q', 'idx_c', 'r_row', 'invc_col'):
            continue  # loaded per-b below
        t_ = consts.tile(shape, dt, tag=f'c_{name}')
        nc.sync.dma_start(t_[:], dram[name][:])
        cs[name] = t_
    eps_t = consts.tile([128, 1], F)
    nc.gpsimd.memset(eps_t[:], EPS)
    ident2 = consts.tile([2, 2], F)
    make_identity(nc, ident2[:])
    ident128 = consts.tile([128, 128], F)
    make_identity(nc, ident128[:])

    # per-b row tiles ([1, 512]) for r at partition 0; inv_c as per-partition
    # columns [128, 4] per b (diagonal-rhs trick scales the c transpose)
    r_tiles, ic_tiles = [], []
    for b in range(BL):
        rt = consts.tile([1, T], F, tag=f'r_{b}')
        nc.sync.dma_start(rt[:], dram['r_row'][b:b + 1, :])
        r_tiles.append(rt)
        it = consts.tile([128, 4], F, tag=f'ic_{b}')
        nc.sync.dma_start(it[:], dram['invc_col'][b, :, :])
        ic_tiles.append(it)

    zsum_all = sbD.tile([2, BL * T], F)
    q_all = sbD.tile([2, BL * T], F)

    for b in range(BL):
        # ===== phase A: embeddings + w/e/a =====
        idxq_b = sbA.tile([128, 4], I32, tag='idxq')
        nc.sync.dma_start(idxq_b[:], dram['idx_q'][b, :, :])
        idxc_b = sbA.tile([128, 16], I32, tag='idxc')
        nc.sync.dma_start(idxc_b[:], dram['idx_c'][b, :, :])
        qg = sbA.tile([128, 4 * D], F, tag='qg')
        for j in range(4):
            nc.gpsimd.indirect_dma_start(
                out=qg[:, j * D:(j + 1) * D], out_offset=None,
                in_=dram['q_tab'][:],
                in_offset=bass.IndirectOffsetOnAxis(
                    ap=idxq_b[:, j:j + 1], axis=0))
        cg = sbA.tile([128, 16 * D], F, tag='cg')
        for kj in range(16):
            nc.gpsimd.indirect_dma_start(
                out=cg[:, kj * D:(kj + 1) * D], out_offset=None,
                in_=dram['c_tab'][:],
                in_offset=bass.IndirectOffsetOnAxis(
                    ap=idxc_b[:, kj:kj + 1], axis=0))

        # diagonal rhs for the c transpose: diag(inv_c) per 128-token block
        diag = sbA.tile([128, 4 * 128], F, tag='diag')
        for j in range(4):
            nc.gpsimd.tensor_scalar_mul(diag[:, j * 128:(j + 1) * 128],
                                        ident128[:], ic_tiles[b][:, j:j + 1])

        # x = [q_e ; c_e] feature-major, built in one [128, T] PSUM tile:
        # q transposes into partitions 0:64, inv_c-scaled c transposes into
        # partitions 64:128 (both quadrant-aligned for the PE).
        x_ps = psA.tile([128, T], F, tag='psa')
        for j in range(4):
            nc.tensor.matmul(x_ps[0:D, 128 * j:128 * (j + 1)],
                             qg[:, j * D:(j + 1) * D],
                             ident128[:], start=True, stop=True)
            for k in range(K):
                nc.tensor.matmul(x_ps[D:2 * D, 128 * j:128 * (j + 1)],
                                 cg[:, (k * 4 + j) * D:(k * 4 + j + 1) * D],
                                 diag[:, j * 128:(j + 1) * 128],
                                 start=(k == 0), stop=(k == K - 1))
        x = sbA.tile([128, T], F, tag='x')
        nc.scalar.copy(x[:], x_ps[:])

        rr_ps = psA.tile([128, T], F, tag='psa')
        nc.tensor.matmul(rr_ps[:], cs['ones1'][:], r_tiles[b][:],
                         start=True, stop=True)
        x_r = sbA.tile([128, T], F, tag='x_r')
        nc.vector.tensor_mul(x_r[:], x[:], rr_ps[:])

        k_ps = psA.tile([D, T], F, tag='psa')
        nc.tensor.matmul(k_ps[:], cs['Wk'][:], x[:], start=True, stop=True)
        kT = sbA.tile([D + 1, T], F, tag='kT')
        nc.gpsimd.memset(kT[D:D + 1, :], 1.0)
        nc.scalar.activation(kT[0:D, :], k_ps[:], AF.Identity,
                             bias=cs['biases'][:, 0:1], scale=1.0)

        v_ps = psA.tile([D, T], F, tag='psa')
        nc.tensor.matmul(v_ps[:], cs['Wv_bot'][:], x[:], start=True, stop=False)
        nc.tensor.matmul(v_ps[:], cs['Wv_dif'][:], x_r[:], start=False, stop=True)
        vT = sbA.tile([D, T], F, tag='vT')
        nc.scalar.activation(vT[:], v_ps[:], AF.Identity,
                             bias=cs['biases'][:, 1:2], scale=1.0)

        e_ps = psA.tile([D, T], F, tag='psa')
        nc.tensor.matmul(e_ps[:], cs['We'][:], vT[:], start=True, stop=True)
        eT = sbA.tile([D, T], F, tag='eT')
        nc.scalar.activation(eT[:], e_ps[:], AF.Sigmoid,
                             bias=cs['biases'][:, 2:3], scale=1.0)
        a_ps = psA.tile([D, T], F, tag='psa')
        nc.tensor.matmul(a_ps[:], cs['Wa'][:], vT[:], start=True, stop=True)
        aT = sbA.tile([D, T], F, tag='aT')
        nc.scalar.activation(aT[:], a_ps[:], AF.Tanh,
                             bias=cs['biases'][:, 3:4], scale=1.0)

        wl_ps = psA.tile([M, T], F, tag='psa')
        nc.tensor.matmul(wl_ps[:], cs['MkT'][:], kT[0:D, :], start=True, stop=True)
        expw = sbA.tile([M, T], F, tag='expw')
        nc.scalar.activation(expw[:], wl_ps[:], AF.Exp)
        sums_ps = psA.tile([1, T], F, tag='psa')
        nc.tensor.matmul(sums_ps[:], cs['ones128'][0:M, :], expw[:],
                         start=True, stop=True)
        lse = sbA.tile([1, T], F, tag='lse')
        nc.scalar.activation(lse[:], sums_ps[:], AF.Ln)
        einv = sbA.tile([1, T], F, tag='einv')
        nc.scalar.activation(einv[:], lse[:], AF.Exp, bias=0.0, scale=-1.0)
        er_ps = psA.tile([M, T], F, tag='psa')
        nc.tensor.matmul(er_ps[:], cs['ones1'][:, 0:M], einv[:],
                         start=True, stop=True)
        wT = sbA.tile([M, T], F, tag='wT')
        nc.vector.tensor_mul(wT[:], expw[:], er_ps[:])

        erep_ps = psA.tile([128, T], F, tag='psa')
        nc.tensor.matmul(erep_ps[:], cs['E64'][:], eT[:], start=True, stop=True)
        e_rep = sbA.tile([128, T], F, tag='e_rep')
        nc.scalar.copy(e_rep[:], erep_ps[:])
        arep_ps = psA.tile([128, T], F, tag='psa')
        nc.tensor.matmul(arep_ps[:], cs['E64'][:], aT[:], start=True, stop=True)
        a_rep = sbA.tile([128, T], F, tag='a_rep')
        nc.scalar.copy(a_rep[:], arep_ps[:])

        # ===== phase B: the recurrence, 25 chunks =====
        read_ps = psR.tile([D, T], F, tag='read')
        for ci in range(NCHUNK):
            on_gp = ci < GP_CHUNKS
            wr_ps = psW.tile([128, T], F, tag='wrep')
            nc.tensor.matmul(wr_ps[:], cs['SelW'][:, ci * 128:(ci + 1) * 128],
                             wT[:], start=True, stop=True)
            w_rep = sbB.tile([128, T], F, tag='w_rep')
            nc.scalar.copy(w_rep[:], wr_ps[:])
            we = sbB.tile([128, T], F, tag='we')
            eng = nc.gpsimd if on_gp else nc.vector
            eng.tensor_mul(we[:], w_rep[:], e_rep[:])
            alpha = sbB.tile([128, T], F, tag='alpha')
            nc.scalar.activation(alpha[:], we[:], AF.Copy, bias=1.0, scale=-1.0)
            beta = sbB.tile([128, T], F, tag='beta')
            nc.gpsimd.tensor_mul(beta[:], w_rep[:], a_rep[:])
            u = sbB.tile([128, T + 1], F, tag='u')
            nc.scalar.copy(u[:, 0:1], cs['Mv0c'][:, ci:ci + 1])
            eng.tensor_tensor_scan(u[:, 1:T + 1], alpha[:], beta[:],
                                   cs['Mv0c'][:, ci:ci + 1], OP.mult, OP.add)
            tmat = sbB.tile([128, T], F, tag='tmat')
            eng.tensor_mul(tmat[:], w_rep[:], u[:, 0:T])
            nc.tensor.matmul(read_ps[:], cs['E64T'][:], tmat[:],
                             start=(ci == 0), stop=(ci == NCHUNK - 1))

        # ===== phase C: heads =====
        rvT = sbA.tile([D, T], F, tag='rvT')
        nc.scalar.copy(rvT[:], read_ps[:])
        f_ps = psA.tile([D, T], F, tag='psa')
        nc.tensor.matmul(f_ps[:], cs['Wf_top'][:], rvT[:], start=True, stop=False)
        nc.tensor.matmul(f_ps[:], cs['Wf_bot'][:], kT[0:D, :], start=False, stop=True)
        fT = sbA.tile([D + 1, T], F, tag='fT')
        nc.gpsimd.memset(fT[D:D + 1, :], 1.0)
        nc.scalar.activation(fT[0:D, :], f_ps[:], AF.Tanh,
                             bias=cs['biases'][:, 4:5], scale=1.0)
        zs_ps = psA.tile([2, T], F, tag='psa')
        nc.tensor.matmul(zs_ps[:], cs['lhsT_p'][:], fT[:], start=True, stop=False)
        nc.tensor.matmul(zs_ps[:], cs['Ws_h'][:], rvT[:], start=False, stop=False)
        nc.tensor.matmul(zs_ps[:], cs['Wq_h'][:], kT[0:D, :], start=False, stop=True)
        nc.scalar.copy(zsum_all[:, b * T:(b + 1) * T], zs_ps[:])
        qh_ps = psA.tile([2, T], F, tag='psa')
        nc.tensor.matmul(qh_ps[:], cs['lhsT_qq'][:], kT[:], start=True, stop=True)
        nc.scalar.copy(q_all[:, b * T:(b + 1) * T], qh_ps[:])

    # ===== phase D: transposed token math + losses =====
    zT_ps = psW.tile([128, BL * 8], F, tag='wrep')
    qT_ps = psW.tile([128, BL * 8], F, tag='wrep')
    for b in range(BL):
        for j in range(4):
            col = b * 8 + 2 * j
            nc.tensor.matmul(zT_ps[:, col:col + 2],
                             zsum_all[:, b * T + 128 * j: b * T + 128 * (j + 1)],
                             ident2[:], start=True, stop=True)
            nc.tensor.matmul(qT_ps[:, col:col + 2],
                             q_all[:, b * T + 128 * j: b * T + 128 * (j + 1)],
                             ident2[:], start=True, stop=True)
    NT = BL * 4    # 32 token-tile columns
    sd = sbD
    qraw = sd.tile([128, 2 * NT], F)
    nc.scalar.copy(qraw[:], qT_ps[:])
    sigz = sd.tile([128, 2 * NT], F)
    nc.scalar.activation(sigz[:], zT_ps[:], AF.Sigmoid)
    z_qks = sd.tile([128, 2 * NT], F)
    nc.scalar.activation(z_qks[:], sigz[:], AF.Ln, bias=eps_t[:], scale=1.0)
    sigq = sd.tile([128, 2 * NT], F)
    nc.scalar.activation(sigq[:], qT_ps[:], AF.Sigmoid)
    z_q = sd.tile([128, 2 * NT], F)
    nc.scalar.activation(z_q[:], sigq[:], AF.Ln, bias=eps_t[:], scale=1.0)

    def coldiff(name, src):
        t_ = sd.tile([128, NT], F, tag=name)
        nc.vector.tensor_sub(t_[:], src[:, 1::2], src[:, 0::2])
        return t_

    d_qks = coldiff('d_qks', z_qks)
    d_q = coldiff('d_q', z_q)
    d_qraw = coldiff('d_qraw', qraw)
    d_core = sd.tile([128, NT], F)
    nc.vector.tensor_sub(d_core[:], d_qks[:], d_q[:])
    pred_sb = sd.tile([128, NT], F)
    nc.scalar.activation(pred_sb[:], d_core[:], AF.Sigmoid)
    nc.sync.dma_start(dram['pred_o'][:], pred_sb[:])

    sigd = sd.tile([128, NT], F)
    nc.scalar.activation(sigd[:], d_qks[:], AF.Sigmoid)
    sdqr = sd.tile([128, NT], F)
    nc.vector.tensor_mul(sdqr[:], d_qraw[:], cs['s_all'][:])
    sdqk = sd.tile([128, NT], F)
    nc.vector.tensor_mul(sdqk[:], d_qks[:], cs['s_all'][:])

    def softplus(name, src, scale):
        e_ = sd.tile([128, NT], F, tag=name + '_e')
        nc.scalar.activation(e_[:], src[:], AF.Exp, bias=0.0, scale=scale)
        s_ = sd.tile([128, NT], F, tag=name + '_s')
        nc.scalar.activation(s_[:], e_[:], AF.Ln, bias=1.0, scale=1.0)
        return s_

    sp1 = softplus('sp1', sdqr, -1.0)
    sp2 = softplus('sp2', sdqk, -1.0)
    sp3 = softplus('sp3', d_q, 1.0)
    t4 = sd.tile([128, NT], F)
    nc.vector.tensor_mul(t4[:], sigd[:], d_q[:])
    u1 = sd.tile([128, NT], F)
    nc.vector.tensor_add(u1[:], sp1[:], sp2[:])
    u2 = sd.tile([128, NT], F)
    nc.vector.tensor_sub(u2[:], sp3[:], t4[:])
    tot = sd.tile([128, NT], F)
    nc.vector.tensor_add(tot[:], u1[:], u2[:])
    scr = sd.tile([128, NT], F)
    acc = sd.tile([128, 1], F)
    nc.vector.scalar_tensor_tensor(scr[:], tot[:], 1.0, cs['mask_all'][:],
                                   OP.bypass, OP.mult, accum_out=acc[:])
    lp_ps = psA.tile([1, 1], F, tag='psa1')
    nc.tensor.matmul(lp_ps[:], cs['ones128'][:], acc[:], start=True, stop=True)
    loss_sb = sd.tile([1, 1], F)
    nc.scalar.copy(loss_sb[:], lp_ps[:])
    nc.sync.dma_start(dram['loss_o'][:], loss_sb[:])


def build_nc(shared):
    nc = bacc.Bacc('TRN2', target_bir_lowering=False, debug=False,
                   num_devices=NCORE)
    dram = {}
    for name, shape, dt in SHARED_SPECS:
        shp = list(shared[name].shape) if shape is None else shape
        dram[name] = nc.dram_tensor(name, shp, dt, kind='ExternalInput').ap()
    for name, shape, dt in PER_CORE_SPECS:
        dram[name] = nc.dram_tensor(name, shape, dt, kind='ExternalInput').ap()
    dram['pred_o'] = nc.dram_tensor('pred_o', [128, BL * 4], F,
                                    kind='ExternalOutput').ap()
    dram['loss_o'] = nc.dram_tensor('loss_o', [1, 1], F,
                                    kind='ExternalOutput').ap()
    with tile.TileContext(nc) as tc:
        with ExitStack() as ctx:
            build_program(ctx, tc, dram)
    nc.compile()
    return nc


_CACHE = {}


def _in_maps(shared, per_core):
    maps = []
    for c in range(NCORE):
        m = {}
        for name, shape, dt in SHARED_SPECS:
            m[name] = np.ascontiguousarray(shared[name])
        for name, shape, dt in PER_CORE_SPECS:
            arr = per_core[c][name]
            m[name] = np.ascontiguousarray(arr.reshape(shape))
        maps.append(m)
    return maps


def assemble(results, t_out, qf_out):
    preds = np.zeros((B, T), np.float32)
    loss_sum = 0.0
    for c in range(NCORE):
        pt = results[c]['pred_o']
        loss_sum += float(results[c]['loss_o'][0, 0])
        for b in range(BL):
            for j in range(4):
                preds[c * BL + b, 128 * j:128 * (j + 1)] = pt[:, b * 4 + j]
    loss = np.float32(loss_sum / (B * L))
    pred = preds[:, 1:].reshape(-1).astype(np.float32)
    return loss, pred, t_out, qf_out


def kernel(**inputs):
    shared, per_core, t_out, qf_out = host_prep(inputs)
    if 'nc' not in _CACHE:
        _CACHE['nc'] = build_nc(shared)
    nc = _CACHE['nc']
    res = run_bass_kernel_spmd(nc, _in_maps(shared, per_core),
                               list(range(NCORE)))
    return assemble(res.results, t_out, qf_out)


# revision 14
# speedup vs baseline: 54.4376x; 54.4376x over previous
"""DKVMN forward kernel for 8 Trainium2 NeuronCores (Bass/Tile).

Self-contained: takes the full un-sharded inputs of the reference
`setup_inputs()`, shards batch-parallel across 8 cores, runs one Bass/Tile
program per core, and assembles the reference's 4 outputs.
"""
import sys

sys.path.insert(0, '/opt/trn_rl_repo')

from contextlib import ExitStack

import numpy as np

import concourse.bacc as bacc
import concourse.bass as bass
import concourse.tile as tile
from concourse import mybir
from concourse.bass_utils import run_bass_kernel_spmd
from concourse.masks import make_identity

F = mybir.dt.float32
I32 = mybir.dt.int32
AF = mybir.ActivationFunctionType
OP = mybir.AluOpType

B, L, K, D, M = 64, 511, 4, 64, 50
T = L + 1            # 512
NCORE = 8
BL = B // NCORE      # 8 batch elements per core
NCHUNK = (M * D) // 128   # 25
EPS = 1e-12

# how many of the 25 chunks per batch element run entirely on GPSIMD
# (load-balancing the DVE); the rest run we/scan/tmat on DVE with beta on GP
GP_CHUNKS = 0


def host_prep(inputs):
    f32 = np.float32
    qseqs = np.asarray(inputs['qseqs']); cseqs = np.asarray(inputs['cseqs'])
    rseqs = np.asarray(inputs['rseqs'])
    sq = np.asarray(inputs['shft_qseqs']); sc = np.asarray(inputs['shft_cseqs'])
    sr = np.asarray(inputs['shft_rseqs'])
    cq = np.concatenate([qseqs[:, :1], sq], 1).astype(np.int64)
    cc = np.concatenate([cseqs[:, :1, :], sc], 1).astype(np.int64)
    cr = np.concatenate([rseqs[:, :1], sr], 1)
    c_num = np.maximum((cc >= 0).sum(-1), 1)
    inv_c = (1.0 / c_num).astype(f32)
    r = cr.astype(f32)

    Wk = np.asarray(inputs['Wk'], f32); bk = np.asarray(inputs['bk'], f32)
    Wv = np.asarray(inputs['Wv'], f32); bv = np.asarray(inputs['bv'], f32)
    Mk = np.asarray(inputs['Mk'], f32); Mv0 = np.asarray(inputs['Mv0'], f32)
    We = np.asarray(inputs['We'], f32); be = np.asarray(inputs['be'], f32)
    Wa = np.asarray(inputs['Wa'], f32); ba = np.asarray(inputs['ba'], f32)
    Wf = np.asarray(inputs['Wf'], f32); bf = np.asarray(inputs['bf'], f32)
    Wp = np.asarray(inputs['Wp'], f32); bp = np.asarray(inputs['bp'], f32)
    Wq = np.asarray(inputs['Wq'], f32); bq = np.asarray(inputs['bq'], f32)
    Ws = np.asarray(inputs['Ws'], f32); bs = np.asarray(inputs['bs'], f32)
    c_tab = np.ascontiguousarray(np.asarray(inputs['c_emb_table'], f32))
    q_tab = np.ascontiguousarray(np.asarray(inputs['q_emb_table'], f32))
    const = np.float32(np.asarray(inputs['constant']))
    NQ = q_tab.shape[0]

    SelW = np.zeros((M, NCHUNK * 128), f32)
    for ci in range(NCHUNK):
        for half in range(2):
            SelW[2 * ci + half,
                 ci * 128 + half * 64: ci * 128 + (half + 1) * 64] = 1.0
    E64 = np.zeros((64, 128), f32)
    for p in range(128):
        E64[p % 64, p] = 1.0

    shared = dict(
        q_tab=q_tab, c_tab=c_tab,
        Wk=Wk, Wv_bot=np.ascontiguousarray(Wv[2 * D:]),
        Wv_dif=np.ascontiguousarray(Wv[:2 * D] - Wv[2 * D:]).astype(f32),
        MkT=np.ascontiguousarray(Mk.T), We=We, Wa=Wa,
        Wf_top=np.ascontiguousarray(Wf[:D]),
        Wf_bot=np.ascontiguousarray(Wf[D:]),
        lhsT_p=np.vstack([Wp, (bp + bq + bs)[None]]).astype(f32),
        Ws_h=Ws, Wq_h=Wq,
        lhsT_qq=np.vstack([Wq, (bq + 2.0 * const)[None]]).astype(f32),
        biases=np.stack([bk, bv, be, ba, bf], 1).astype(f32),
        Mv0c=np.ascontiguousarray(Mv0.reshape(M * D).reshape(NCHUNK, 128).T),
        SelW=SelW, E64=E64,
        E64T=np.ascontiguousarray(E64.T),
        ones128=np.ones((128, 1), f32),
        ones1=np.ones((1, 128), f32),
    )

    per_core = []
    for c in range(NCORE):
        bsl = slice(c * BL, (c + 1) * BL)
        cq_c, cc_c = cq[bsl], cc[bsl]
        sr_c = np.asarray(sr[bsl])
        idx_q = np.zeros((BL, 128, 4), np.int32)
        idx_c = np.zeros((BL, 128, 16), np.int32)
        for b in range(BL):
            for j in range(4):
                toks = np.arange(128) + 128 * j
                idx_q[b, :, j] = cq_c[b, toks]
                for k in range(K):
                    idx_c[b, :, k * 4 + j] = cc_c[b, toks, k] + 1
        s_all = np.zeros((128, 32), f32)
        mask = np.zeros((128, 32), f32)
        invc_col = np.zeros((BL, 128, 4), f32)
        for b in range(BL):
            for j in range(4):
                toks = np.arange(128) + 128 * j
                valid = toks >= 1
                lab = np.zeros(128, np.int64)
                lab[valid] = sr_c[b, toks[valid] - 1]
                s_all[:, b * 4 + j] = 2.0 * lab - 1.0
                mask[:, b * 4 + j] = valid.astype(f32)
                invc_col[b, :, j] = inv_c[c * BL + b, toks]
        per_core.append(dict(
            idx_q=idx_q, idx_c=idx_c,
            r_row=r[bsl].astype(f32), invc_col=invc_col,
            s_all=s_all, mask_all=mask,
        ))

    t_out = sr.reshape(-1).astype(np.int32)
    qf_out = (np.asarray(sq).reshape(-1).astype(np.int64)
              + NQ * sr.reshape(-1).astype(np.int64)).astype(np.int32)
    return shared, per_core, t_out, qf_out


SHARED_SPECS = [
    ('q_tab', None, F), ('c_tab', None, F),
    ('Wk', [2 * D, D], F), ('Wv_bot', [2 * D, D], F), ('Wv_dif', [2 * D, D], F),
    ('MkT', [D, M], F), ('We', [D, D], F), ('Wa', [D, D], F),
    ('Wf_top', [D, D], F), ('Wf_bot', [D, D], F),
    ('lhsT_p', [D + 1, 2], F), ('Ws_h', [D, 2], F), ('Wq_h', [D, 2], F),
    ('lhsT_qq', [D + 1, 2], F),
    ('biases', [D, 5], F), ('Mv0c', [128, NCHUNK], F),
    ('SelW', [M, NCHUNK * 128], F), ('E64', [D, 128], F), ('E64T', [128, D], F),
    ('ones128', [128, 1], F), ('ones1', [1, 128], F),
]
PER_CORE_SPECS = [
    ('idx_q', [BL, 128, 4], I32), ('idx_c', [BL, 128, 16], I32),
    ('r_row', [BL, T], F), ('invc_col', [BL, 128, 4], F),
    ('s_all', [128, 32], F), ('mask_all', [128, 32], F),
]


def build_program(ctx: ExitStack, tc: tile.TileContext, dram, reps=1):
    nc = tc.nc

    consts = ctx.enter_context(tc.tile_pool(name='consts', bufs=1))
    sbA = ctx.enter_context(tc.tile_pool(name='sbA', bufs=2))
    sbB = ctx.enter_context(tc.tile_pool(name='sbB', bufs=3))
    sbD = ctx.enter_context(tc.tile_pool(name='sbD', bufs=1))
    psA = ctx.enter_context(tc.tile_pool(name='psA', bufs=2, space='PSUM'))
    psW = ctx.enter_context(tc.tile_pool(name='psW', bufs=2, space='PSUM'))
    psR = ctx.enter_context(tc.tile_pool(name='psR', bufs=2, space='PSUM'))

    # ---- load constants/weights to SBUF
    cs = {}
    for name, shape, dt in SHARED_SPECS:
        if name in ('q_tab', 'c_tab'):
            continue
        t_ = consts.tile(shape, dt, tag=f'c_{name}')
        nc.sync.dma_start(t_[:], dram[name][:])
        cs[name] = t_
    for name, shape, dt in PER_CORE_SPECS:
        if name in ('idx_q', 'idx_c', 'r_row', 'invc_col'):
            continue  # loaded per-b below
        t_ = consts.tile(shape, dt, tag=f'c_{name}')
        nc.sync.dma_start(t_[:], dram[name][:])
        cs[name] = t_
    eps_t = consts.tile([128, 1], F)
    nc.gpsimd.memset(eps_t[:], EPS)
    ident2 = consts.tile([2, 2], F)
    make_identity(nc, ident2[:])
    ident128 = consts.tile([128, 128], F)
    make_identity(nc, ident128[:])

    # per-b row tiles ([1, 512]) for r at partition 0; inv_c as per-partition
    # columns [128, 4] per b (diagonal-rhs trick scales the c transpose)
    r_tiles, ic_tiles = [], []
    for b in range(BL):
        rt = consts.tile([1, T], F, tag=f'r_{b}')
        nc.sync.dma_start(rt[:], dram['r_row'][b:b + 1, :])
        r_tiles.append(rt)
        it = consts.tile([128, 4], F, tag=f'ic_{b}')
        nc.sync.dma_start(it[:], dram['invc_col'][b, :, :])
        ic_tiles.append(it)

    zsum_all = sbD.tile([2, BL * T], F)
    q_all = sbD.tile([2, BL * T], F)

    rep_ctx = tc.For_i(0, reps, 1) if reps > 1 else None
    if rep_ctx is not None:
        ctx.enter_context(rep_ctx)

    for b in range(BL):
        # ===== phase A: embeddings + w/e/a =====
        idxq_b = sbA.tile([128, 4], I32, tag='idxq')
        nc.sync.dma_start(idxq_b[:], dram['idx_q'][b, :, :])
        idxc_b = sbA.tile([128, 16], I32, tag='idxc')
        nc.sync.dma_start(idxc_b[:], dram['idx_c'][b, :, :])
        qg = sbA.tile([128, 4 * D], F, tag='qg')
        for j in range(4):
            nc.gpsimd.indirect_dma_start(
                out=qg[:, j * D:(j + 1) * D], out_offset=None,
                in_=dram['q_tab'][:],
                in_offset=bass.IndirectOffsetOnAxis(
                    ap=idxq_b[:, j:j + 1], axis=0))
        cg = sbA.tile([128, 16 * D], F, tag='cg')
        for kj in range(16):
            nc.gpsimd.indirect_dma_start(
                out=cg[:, kj * D:(kj + 1) * D], out_offset=None,
                in_=dram['c_tab'][:],
                in_offset=bass.IndirectOffsetOnAxis(
                    ap=idxc_b[:, kj:kj + 1], axis=0))

        # diagonal rhs for the c transpose: diag(inv_c) per 128-token block
        diag = sbA.tile([128, 4 * 128], F, tag='diag')
        for j in range(4):
            nc.gpsimd.tensor_scalar_mul(diag[:, j * 128:(j + 1) * 128],
                                        ident128[:], ic_tiles[b][:, j:j + 1])

        # x = [q_e ; c_e] feature-major, built in one [128, T] PSUM tile:
        # q transposes into partitions 0:64, inv_c-scaled c transposes into
        # partitions 64:128 (both quadrant-aligned for the PE).
        x_ps = psA.tile([128, T], F, tag='psa')
        for j in range(4):
            nc.tensor.matmul(x_ps[0:D, 128 * j:128 * (j + 1)],
                             qg[:, j * D:(j + 1) * D],
                             ident128[:], start=True, stop=True)
            for k in range(K):
                nc.tensor.matmul(x_ps[D:2 * D, 128 * j:128 * (j + 1)],
                                 cg[:, (k * 4 + j) * D:(k * 4 + j + 1) * D],
                                 diag[:, j * 128:(j + 1) * 128],
                                 start=(k == 0), stop=(k == K - 1))
        x = sbA.tile([128, T], F, tag='x')
        nc.scalar.copy(x[:], x_ps[:])

        rr_ps = psA.tile([128, T], F, tag='psa')
        nc.tensor.matmul(rr_ps[:], cs['ones1'][:], r_tiles[b][:],
                         start=True, stop=True)
        x_r = sbA.tile([128, T], F, tag='x_r')
        nc.vector.tensor_mul(x_r[:], x[:], rr_ps[:])

        k_ps = psA.tile([D, T], F, tag='psa')
        nc.tensor.matmul(k_ps[:], cs['Wk'][:], x[:], start=True, stop=True)
        kT = sbA.tile([D + 1, T], F, tag='kT')
        nc.gpsimd.memset(kT[D:D + 1, :], 1.0)
        nc.scalar.activation(kT[0:D, :], k_ps[:], AF.Identity,
                             bias=cs['biases'][:, 0:1], scale=1.0)

        v_ps = psA.tile([D, T], F, tag='psa')
        nc.tensor.matmul(v_ps[:], cs['Wv_bot'][:], x[:], start=True, stop=False)
        nc.tensor.matmul(v_ps[:], cs['Wv_dif'][:], x_r[:], start=False, stop=True)
        vT = sbA.tile([D, T], F, tag='vT')
        nc.scalar.activation(vT[:], v_ps[:], AF.Identity,
                             bias=cs['biases'][:, 1:2], scale=1.0)

        e_ps = psA.tile([D, T], F, tag='psa')
        nc.tensor.matmul(e_ps[:], cs['We'][:], vT[:], start=True, stop=True)
        eT = sbA.tile([D, T], F, tag='eT')
        nc.scalar.activation(eT[:], e_ps[:], AF.Sigmoid,
                             bias=cs['biases'][:, 2:3], scale=1.0)
        a_ps = psA.tile([D, T], F, tag='psa')
        nc.tensor.matmul(a_ps[:], cs['Wa'][:], vT[:], start=True, stop=True)
        aT = sbA.tile([D, T], F, tag='aT')
        nc.scalar.activation(aT[:], a_ps[:], AF.Tanh,
                             bias=cs['biases'][:, 3:4], scale=1.0)

        wl_ps = psA.tile([M, T], F, tag='psa')
        nc.tensor.matmul(wl_ps[:], cs['MkT'][:], kT[0:D, :], start=True, stop=True)
        expw = sbA.tile([M, T], F, tag='expw')
        nc.scalar.activation(expw[:], wl_ps[:], AF.Exp)
        sums_ps = psA.tile([1, T], F, tag='psa')
        nc.tensor.matmul(sums_ps[:], cs['ones128'][0:M, :], expw[:],
                         start=True, stop=True)
        lse = sbA.tile([1, T], F, tag='lse')
        nc.scalar.activation(lse[:], sums_ps[:], AF.Ln)
        einv = sbA.tile([1, T], F, tag='einv')
        nc.scalar.activation(einv[:], lse[:], AF.Exp, bias=0.0, scale=-1.0)
        er_ps = psA.tile([M, T], F, tag='psa')
        nc.tensor.matmul(er_ps[:], cs['ones1'][:, 0:M], einv[:],
                         start=True, stop=True)
        wT = sbA.tile([M, T], F, tag='wT')
        nc.vector.tensor_mul(wT[:], expw[:], er_ps[:])

        erep_ps = psA.tile([128, T], F, tag='psa')
        nc.tensor.matmul(erep_ps[:], cs['E64'][:], eT[:], start=True, stop=True)
        e_rep = sbA.tile([128, T], F, tag='e_rep')
        nc.scalar.copy(e_rep[:], erep_ps[:])
        arep_ps = psA.tile([128, T], F, tag='psa')
        nc.tensor.matmul(arep_ps[:], cs['E64'][:], aT[:], start=True, stop=True)
        a_rep = sbA.tile([128, T], F, tag='a_rep')
        nc.scalar.copy(a_rep[:], arep_ps[:])

        # ===== phase B: the recurrence, 25 chunks =====
        read_ps = psR.tile([D, T], F, tag='read')
        for ci in range(NCHUNK):
            on_gp = ci < GP_CHUNKS
            wr_ps = psW.tile([128, T], F, tag='wrep')
            nc.tensor.matmul(wr_ps[:], cs['SelW'][:, ci * 128:(ci + 1) * 128],
                             wT[:], start=True, stop=True)
            w_rep = sbB.tile([128, T], F, tag='w_rep')
            nc.scalar.copy(w_rep[:], wr_ps[:])
            we = sbB.tile([128, T], F, tag='we')
            eng = nc.gpsimd if on_gp else nc.vector
            eng.tensor_mul(we[:], w_rep[:], e_rep[:])
            alpha = sbB.tile([128, T], F, tag='alpha')
            nc.scalar.activation(alpha[:], we[:], AF.Copy, bias=1.0, scale=-1.0)
            beta = sbB.tile([128, T], F, tag='beta')
            nc.gpsimd.tensor_mul(beta[:], w_rep[:], a_rep[:])
            u = sbB.tile([128, T + 1], F, tag='u')
            nc.scalar.copy(u[:, 0:1], cs['Mv0c'][:, ci:ci + 1])
            eng.tensor_tensor_scan(u[:, 1:T + 1], alpha[:], beta[:],
                                   cs['Mv0c'][:, ci:ci + 1], OP.mult, OP.add)
            tmat = sbB.tile([128, T], F, tag='tmat')
            eng.tensor_mul(tmat[:], w_rep[:], u[:, 0:T])
            nc.tensor.matmul(read_ps[:], cs['E64T'][:], tmat[:],
                             start=(ci == 0), stop=(ci == NCHUNK - 1))

        # ===== phase C: heads =====
        rvT = sbA.tile([D, T], F, tag='rvT')
        nc.scalar.copy(rvT[:], read_ps[:])
        f_ps = psA.tile([D, T], F, tag='psa')
        nc.tensor.matmul(f_ps[:], cs['Wf_top'][:], rvT[:], start=True, stop=False)
        nc.tensor.matmul(f_ps[:], cs['Wf_bot'][:], kT[0:D, :], start=False, stop=True)
        fT = sbA.tile([D + 1, T], F, tag='fT')
        nc.gpsimd.memset(fT[D:D + 1, :], 1.0)
        nc.scalar.activation(fT[0:D, :], f_ps[:], AF.Tanh,
                             bias=cs['biases'][:, 4:5], scale=1.0)
        zs_ps = psA.tile([2, T], F, tag='psa')
        nc.tensor.matmul(zs_ps[:], cs['lhsT_p'][:], fT[:], start=True, stop=False)
        nc.tensor.matmul(zs_ps[:], cs['Ws_h'][:], rvT[:], start=False, stop=False)
        nc.tensor.matmul(zs_ps[:], cs['Wq_h'][:], kT[0:D, :], start=False, stop=True)
        nc.scalar.copy(zsum_all[:, b * T:(b + 1) * T], zs_ps[:])
        qh_ps = psA.tile([2, T], F, tag='psa')
        nc.tensor.matmul(qh_ps[:], cs['lhsT_qq'][:], kT[:], start=True, stop=True)
        nc.scalar.copy(q_all[:, b * T:(b + 1) * T], qh_ps[:])

    # ===== phase D: transposed token math + losses =====
    zT_ps = psW.tile([128, BL * 8], F, tag='wrep')
    qT_ps = psW.tile([128, BL * 8], F, tag='wrep')
    for b in range(BL):
        for j in range(4):
            col = b * 8 + 2 * j
            nc.tensor.matmul(zT_ps[:, col:col + 2],
                             zsum_all[:, b * T + 128 * j: b * T + 128 * (j + 1)],
                             ident2[:], start=True, stop=True)
            nc.tensor.matmul(qT_ps[:, col:col + 2],
                             q_all[:, b * T + 128 * j: b * T + 128 * (j + 1)],
                             ident2[:], start=True, stop=True)
    NT = BL * 4    # 32 token-tile columns
    sd = sbD
    qraw = sd.tile([128, 2 * NT], F)
    nc.scalar.copy(qraw[:], qT_ps[:])
    sigz = sd.tile([128, 2 * NT], F)
    nc.scalar.activation(sigz[:], zT_ps[:], AF.Sigmoid)
    z_qks = sd.tile([128, 2 * NT], F)
    nc.scalar.activation(z_qks[:], sigz[:], AF.Ln, bias=eps_t[:], scale=1.0)
    sigq = sd.tile([128, 2 * NT], F)
    nc.scalar.activation(sigq[:], qT_ps[:], AF.Sigmoid)
    z_q = sd.tile([128, 2 * NT], F)
    nc.scalar.activation(z_q[:], sigq[:], AF.Ln, bias=eps_t[:], scale=1.0)

    def coldiff(name, src):
        t_ = sd.tile([128, NT], F, tag=name)
        nc.vector.tensor_sub(t_[:], src[:, 1::2], src[:, 0::2])
        return t_

    d_qks = coldiff('d_qks', z_qks)
    d_q = coldiff('d_q', z_q)
    d_qraw = coldiff('d_qraw', qraw)
    d_core = sd.tile([128, NT], F)
    nc.vector.tensor_sub(d_core[:], d_qks[:], d_q[:])
    pred_sb = sd.tile([128, NT], F)
    nc.scalar.activation(pred_sb[:], d_core[:], AF.Sigmoid)
    nc.sync.dma_start(dram['pred_o'][:], pred_sb[:])

    sigd = sd.tile([128, NT], F)
    nc.scalar.activation(sigd[:], d_qks[:], AF.Sigmoid)
    sdqr = sd.tile([128, NT], F)
    nc.vector.tensor_mul(sdqr[:], d_qraw[:], cs['s_all'][:])
    sdqk = sd.tile([128, NT], F)
    nc.vector.tensor_mul(sdqk[:], d_qks[:], cs['s_all'][:])

    def softplus(name, src, scale):
        e_ = sd.tile([128, NT], F, tag=name + '_e')
        nc.scalar.activation(e_[:], src[:], AF.Exp, bias=0.0, scale=scale)
        s_ = sd.tile([128, NT], F, tag=name + '_s')
        nc.scalar.activation(s_[:], e_[:], AF.Ln, bias=1.0, scale=1.0)
        return s_

    sp1 = softplus('sp1', sdqr, -1.0)
    sp2 = softplus('sp2', sdqk, -1.0)
    sp3 = softplus('sp3', d_q, 1.0)
    t4 = sd.tile([128, NT], F)
    nc.vector.tensor_mul(t4[:], sigd[:], d_q[:])
    u1 = sd.tile([128, NT], F)
    nc.vector.tensor_add(u1[:], sp1[:], sp2[:])
    u2 = sd.tile([128, NT], F)
    nc.vector.tensor_sub(u2[:], sp3[:], t4[:])
    tot = sd.tile([128, NT], F)
    nc.vector.tensor_add(tot[:], u1[:], u2[:])
    scr = sd.tile([128, NT], F)
    acc = sd.tile([128, 1], F)
    nc.vector.scalar_tensor_tensor(scr[:], tot[:], 1.0, cs['mask_all'][:],
                                   OP.bypass, OP.mult, accum_out=acc[:])
    lp_ps = psA.tile([1, 1], F, tag='psa1')
    nc.tensor.matmul(lp_ps[:], cs['ones128'][:], acc[:], start=True, stop=True)
    loss_sb = sd.tile([1, 1], F)
    nc.scalar.copy(loss_sb[:], lp_ps[:])
    nc.sync.dma_start(dram['loss_o'][:], loss_sb[:])


def build_nc(shared, reps=1):
    nc = bacc.Bacc('TRN2', target_bir_lowering=False, debug=False,
                   num_devices=NCORE)
    dram = {}
    for name, shape, dt in SHARED_SPECS:
        shp = list(shared[name].shape) if shape is None else shape
        dram[name] = nc.dram_tensor(name, shp, dt, kind='ExternalInput').ap()
    for name, shape, dt in PER_CORE_SPECS:
        dram[name] = nc.dram_tensor(name, shape, dt, kind='ExternalInput').ap()
    dram['pred_o'] = nc.dram_tensor('pred_o', [128, BL * 4], F,
                                    kind='ExternalOutput').ap()
    dram['loss_o'] = nc.dram_tensor('loss_o', [1, 1], F,
                                    kind='ExternalOutput').ap()
    with tile.TileContext(nc) as tc:
        with ExitStack() as ctx:
            build_program(ctx, tc, dram, reps=reps)
    nc.compile()
    return nc


_CACHE = {}


def _in_maps(shared, per_core):
    maps = []
    for c in range(NCORE):
        m = {}
        for name, shape, dt in SHARED_SPECS:
            m[name] = np.ascontiguousarray(shared[name])
        for name, shape, dt in PER_CORE_SPECS:
            arr = per_core[c][name]
            m[name] = np.ascontiguousarray(arr.reshape(shape))
        maps.append(m)
    return maps


def assemble(results, t_out, qf_out):
    preds = np.zeros((B, T), np.float32)
    loss_sum = 0.0
    for c in range(NCORE):
        pt = results[c]['pred_o']
        loss_sum += float(results[c]['loss_o'][0, 0])
        for b in range(BL):
            for j in range(4):
                preds[c * BL + b, 128 * j:128 * (j + 1)] = pt[:, b * 4 + j]
    loss = np.float32(loss_sum / (B * L))
    pred = preds[:, 1:].reshape(-1).astype(np.float32)
    return loss, pred, t_out, qf_out


def kernel(**inputs):
    shared, per_core, t_out, qf_out = host_prep(inputs)
    if 'nc' not in _CACHE:
        _CACHE['nc'] = build_nc(shared)
    nc = _CACHE['nc']
    res = run_bass_kernel_spmd(nc, _in_maps(shared, per_core),
                               list(range(NCORE)))
    return assemble(res.results, t_out, qf_out)
